# revision 16
# baseline (speedup 1.0000x reference)
"""Windowed multi-head attention (B=128 windows, N=512, C=256, H=8) on 8 TRN2 NeuronCores.

Strategy: data-parallel over windows (16 per core). The softmax exp on the
ScalarE (ACT) engine is the hard bottleneck (8 heads x 512x512 exps per
window = 16384 ACT lane-cycles @1.2GHz ~ 13.7us/window), so the kernel is
built so ACT does *only* exp and is never starved:

  - q/k packed 4 heads per 128-partition block; scores use 4x row-tiled
    matmuls (tile_position=(32h',0), contraction=head_dim=32) so 2-4 score
    matmuls run concurrently in the PE array.
  - exp consumes 2-bank PSUM score sets [128, 2x512] in single ACT calls.
  - AV uses 4x column tiling (stationary = V_h [j,32], moving = P_h) so the
    attention output lands *pre-transposed* [d, q] in one PSUM bank -- no
    PE transposes. A second col-tiled pass with an all-ones [128,32]
    stationary produces the softmax denominator replicated across each
    32-partition group, so normalization is a full-width DVE recip+mul.
  - all PSUM evacuations (qkv bias-add, v bias-add, normalize, proj bias)
    run on VectorE; outputs are written bf16 to halve the store DMA.

Per window, 8 software-pipeline units (g group of 4 heads, jb key block):
unit = 4 row-tiled score matmuls (two 2-bank PSUM sets) + 2 exp calls;
the previous unit's 8 AV/denominator matmuls trail one unit behind
(batched to halve PE mode-switch drains); the next window's qkv matmuls
are interleaved one chunk per unit; the last unit's AV work and the
projection trail into the NEXT window (KERNEL_DEFER) so ACT never idles
at window boundaries.  KERNEL_SCHRAUD=n offloads n units' exp to the
VectorE as a one-instruction Schraudolph bf16 exp (tensor_scalar
round(s*128/ln2 + 16256) -> int16, bitcast bf16), splitting the exp
dependency ladder across two engines at a small accuracy cost.
KERNEL_STPAIR (default on) emits all 4 score matmuls of a unit before
its two exp calls so the second exp reaches the ACT queue head with its
semaphore wait already satisfied.

Measured HW facts (8-core repeat-loop bench, ~±15us/pass run noise):
  - baseline ~392-412us/pass (16 windows/core); ACT exp work is ~1:1 on
    the critical path (EXP_REPS=2 adds 94% of the added ACT time); DVE
    ops cost ~1.8x nominal on the critical path (post-op DRAIN).
  - removing 24/32 denominator matmuls (NO_DN=1) does NOT speed it up:
    the PE AV/dn work trails in slack; only the score->exp chain and
    the two softmax engines are critical.
  - every engine-reassignment tried is worse: SCHRAUD=0 +80us,
    SCHRAUD=8 +214us, EVACT=1 +34us, SET4=1 (single-buffered st) +60us,
    PEND=2 +40us.  SCHRAUD=2 with evacs on DVE is the balance point.
  - PSUM is exactly full (st 2x2 + av/dn 2 + mmout 2 = 8 banks); a 3rd
    st set (which would let ACT and DVE exp overlap across units) does
    not fit, which caps further gains from engine overlap.
"""
import os
import sys

sys.path.insert(0, "/opt/trn_rl_repo")

import numpy as np
import ml_dtypes
from contextlib import ExitStack

N_CORES = 8
B, N, C = 128, 512, 256
H, HD = 8, 32
W = B // N_CORES  # windows per core


def build_nc(n_windows=W, repeat=None):
    import concourse.bass as bass
    import concourse.tile as tile
    from concourse import bacc, mybir

    if repeat is None:
        repeat = int(os.environ.get("KERNEL_REPEAT", "1"))

    F32 = mybir.dt.float32
    BF16 = mybir.dt.bfloat16
    MMDT = BF16
    Exp = mybir.ActivationFunctionType.Exp
    Ident = mybir.ActivationFunctionType.Identity
    # route qkv/proj bias-add evacuations to ACT (Identity shares the exp
    # table set) to free DVE for more Schraudolph exp calls
    evact = os.environ.get("KERNEL_EVACT", "0") == "1"
    # fold the V bias into the proj bias host-side (softmax rows sum to 1,
    # so V-bias contributes exactly proj_w @ v_bias to the output); the V
    # evacuation becomes a pure copy, optionally on ACT
    vfold = os.environ.get("KERNEL_VFOLD", "0") == "1"
    vevac_act = os.environ.get("KERNEL_VEVAC", "dve") == "act"

    nc = bacc.Bacc("TRN2", target_bir_lowering=False, debug=False,
                   num_devices=N_CORES)
    xt_d = nc.dram_tensor("xt", [n_windows, 128, 2, 512], MMDT,
                          kind="ExternalInput").ap()
    wq_d = nc.dram_tensor("wq", [128, 2, 768], MMDT, kind="ExternalInput").ap()
    bq_d = nc.dram_tensor("bq", [128, 4], F32, kind="ExternalInput").ap()
    bv_d = nc.dram_tensor("bv", [128, 256], F32, kind="ExternalInput").ap()
    wp_d = nc.dram_tensor("wp", [128, 2, 256], BF16, kind="ExternalInput").ap()
    bp_d = nc.dram_tensor("bp", [128, 2], F32, kind="ExternalInput").ap()
    ot_d = nc.dram_tensor("ot", [n_windows, 128, 2, 512], BF16,
                          kind="ExternalOutput").ap()

    with tile.TileContext(nc) as tc, ExitStack() as ctx:
        big = os.environ.get("KERNEL_POOLBIG", "0") == "1"
        persist = ctx.enter_context(tc.tile_pool(name="persist", bufs=1))
        xpool = ctx.enter_context(tc.tile_pool(name="xpool", bufs=4 if big else 3))
        qkpool = ctx.enter_context(tc.tile_pool(name="qkpool", bufs=2))
        vpool = ctx.enter_context(tc.tile_pool(name="vpool", bufs=3 if big else 2))
        _pend_bufs = 2 * (int(os.environ.get("KERNEL_PEND", "1")) - 1)
        ppool = ctx.enter_context(tc.tile_pool(
            name="ppool", bufs=(10 if big else 6) + _pend_bufs))
        recpool = ctx.enter_context(tc.tile_pool(name="recpool", bufs=3 if big else 2))
        atpool = ctx.enter_context(tc.tile_pool(name="atpool", bufs=2))
        finpool = ctx.enter_context(tc.tile_pool(name="finpool", bufs=4 if big else 3))
        stpool = ctx.enter_context(tc.tile_pool(
            name="stpool",
            bufs=1 if os.environ.get("KERNEL_SET4", "0") == "1" else 2,
            space="PSUM"))
        avpool = ctx.enter_context(tc.tile_pool(name="avpool", bufs=1, space="PSUM"))
        mmout = ctx.enter_context(tc.tile_pool(name="mmout", bufs=2, space="PSUM"))

        wq_sb = persist.tile([128, 2, 768], MMDT, tag="wq")
        nc.sync.dma_start(out=wq_sb, in_=wq_d)
        bq_sb = persist.tile([128, 4], F32, tag="bq")
        nc.sync.dma_start(out=bq_sb, in_=bq_d)
        bv_sb = persist.tile([128, 256], F32, tag="bv")
        nc.sync.dma_start(out=bv_sb, in_=bv_d)
        wp_sb = persist.tile([128, 2, 256], BF16, tag="wp")
        nc.sync.dma_start(out=wp_sb, in_=wp_d)
        bp_sb = persist.tile([128, 2], F32, tag="bp")
        nc.sync.dma_start(out=bp_sb, in_=bp_d)
        ones = persist.tile([128, 32], BF16, tag="ones")
        nc.gpsimd.memset(ones, 1.0)

        def make_qkv(iv):
            """Emit qkv for window iv as a list of chunk closures."""
            state = {"qk": []}

            def c0():
                xw = xpool.tile([128, 2, 512], MMDT, tag="xw", name="xw")
                nc.sync.dma_start(out=xw, in_=xt_d[iv])
                state["xw"] = xw
                state["vaug"] = vpool.tile([128, 4, 8, 32], BF16, tag="vaug",
                                           name="vaug")

            def cmb(mb):
                def f():
                    ps = mmout.tile([128, 512], F32, tag="mm", name="qkps")
                    xw = state["xw"]
                    for cb in range(2):
                        nc.tensor.matmul(
                            ps, wq_sb[:, cb, 128 * mb:128 * mb + 128],
                            xw[:, cb, :], start=(cb == 0), stop=(cb == 1))
                    qkt = qkpool.tile([128, 512], BF16, tag=f"qk{mb}",
                                      name=f"qk{mb}")
                    if evact:
                        nc.scalar.activation(out=qkt, in_=ps, func=Ident,
                                             bias=bq_sb[:, mb:mb + 1],
                                             scale=1.0)
                    else:
                        nc.vector.tensor_scalar_add(qkt, ps,
                                                    bq_sb[:, mb:mb + 1])
                    state["qk"].append(qkt)
                return f

            def cv(tp):
                def f():
                    ps = mmout.tile([128, 512], F32, tag="mm", name="vps")
                    xw, vaug = state["xw"], state["vaug"]
                    for half in range(2):
                        for cb in range(2):
                            nc.tensor.matmul(
                                ps[:, 256 * half:256 * half + 256],
                                xw[:, cb, 128 * (2 * tp + half):
                                   128 * (2 * tp + half) + 128],
                                wq_sb[:, cb, 512:768],
                                start=(cb == 0), stop=(cb == 1))
                    dst = vaug[:, 2 * tp:2 * tp + 2, :, :]
                    src = ps.rearrange("p (t h d) -> p t h d", t=2, h=8)
                    if vfold:
                        if vevac_act:
                            nc.scalar.copy(dst, src)
                        else:
                            nc.vector.tensor_copy(dst, src)
                    else:
                        bvb = bass.AP(tensor=bv_sb.tensor, offset=bv_sb.offset,
                                      ap=[[bv_sb.ap[0][0], 128], [0, 2],
                                          [32, 8], [1, 32]])
                        nc.vector.tensor_add(dst, src, bvb)
                return f

            return [c0, cmb(0), cmb(1), cmb(2), cmb(3), cv(0), cv(1)], state

        sc_reps = int(os.environ.get("KERNEL_SC_REPS", "1"))
        av_reps = int(os.environ.get("KERNEL_AV_REPS", "1"))
        exp_reps = int(os.environ.get("KERNEL_EXP_REPS", "1"))
        no_dn = os.environ.get("KERNEL_NO_DN", "0") == "1"
        set4 = os.environ.get("KERNEL_SET4", "0") == "1"
        # units per window whose exp runs on DVE (Schraudolph bf16) instead
        # of ACT: splits the exp dependency ladder across two engines
        schraud = int(os.environ.get("KERNEL_SCHRAUD", "2"))
        schraud_jbs = [[], [2], [1, 3], [1, 2, 3], [0, 1, 2, 3]][schraud // 2]
        SCH_A = float(128.0 / np.log(2.0))
        SCH_B = 16256.0
        I16 = mybir.dt.int16

        def emit_st(qk, g, jb, s):
            """Two row-tiled score matmuls + one exp call for head pair."""
            st2 = stpool.tile([128, 2, 512], F32, tag="st", name="st")
            for _ in range(sc_reps):
                for i in range(2):
                    hp = 2 * s + i
                    nc.tensor.matmul(
                        st2[:, i, :],
                        qk[2 + g][32 * hp:32 * hp + 32, 128 * jb:128 * jb + 128],
                        qk[g][32 * hp:32 * hp + 32, :], start=True, stop=True,
                        tile_position=(32 * hp, 0))
            ph = ppool.tile([128, 2, 512], BF16, tag="ph", name="ph")
            if jb in schraud_jbs:
                nc.vector.tensor_scalar(
                    out=ph.bitcast(I16), in0=st2, scalar1=SCH_A, scalar2=SCH_B,
                    op0=mybir.AluOpType.mult, op1=mybir.AluOpType.add)
            else:
                for _ in range(exp_reps):
                    nc.scalar.activation(out=ph, in_=st2, func=Exp)
            return ph

        schmm = os.environ.get("KERNEL_SCHMM", "0") == "1"

        def emit_st_pair(qk, g, jb):
            """All 4 score matmuls first, then both exp calls — halves the
            number of dependent sem hops on the exp stream (the second exp's
            wait is already satisfied when it reaches the queue head).

            KERNEL_SCHMM=1: Schraudolph (DVE) units take their score tiles
            from the mmout pool (1-bank granularity) instead of stpool, so
            the ACT units' st-set ping-pong never passes through a DVE unit
            and the ACT exp stream can run ahead across it."""
            if schmm and jb in schraud_jbs:
                sts = []
                for hp in range(4):
                    st1 = mmout.tile([128, 512], F32, tag="mm", name="sst")
                    for _ in range(sc_reps):
                        nc.tensor.matmul(
                            st1,
                            qk[2 + g][32 * hp:32 * hp + 32,
                                      128 * jb:128 * jb + 128],
                            qk[g][32 * hp:32 * hp + 32, :], start=True,
                            stop=True, tile_position=(32 * hp, 0))
                    sts.append(st1)
                phs = []
                for s in range(2):
                    ph = ppool.tile([128, 2, 512], BF16, tag="ph", name="ph")
                    for i in range(2):
                        nc.vector.tensor_scalar(
                            out=ph[:, i, :].bitcast(I16), in0=sts[2 * s + i],
                            scalar1=SCH_A, scalar2=SCH_B,
                            op0=mybir.AluOpType.mult,
                            op1=mybir.AluOpType.add)
                    phs.append(ph)
                return phs
            sts = []
            for s in range(2):
                st2 = stpool.tile([128, 2, 512], F32, tag="st", name="st")
                for _ in range(sc_reps):
                    for i in range(2):
                        hp = 2 * s + i
                        nc.tensor.matmul(
                            st2[:, i, :],
                            qk[2 + g][32 * hp:32 * hp + 32,
                                      128 * jb:128 * jb + 128],
                            qk[g][32 * hp:32 * hp + 32, :], start=True,
                            stop=True, tile_position=(32 * hp, 0))
                sts.append(st2)
            phs = []
            for s in range(2):
                ph = ppool.tile([128, 2, 512], BF16, tag="ph", name="ph")
                if jb in schraud_jbs:
                    nc.vector.tensor_scalar(
                        out=ph.bitcast(I16), in0=sts[s], scalar1=SCH_A,
                        scalar2=SCH_B, op0=mybir.AluOpType.mult,
                        op1=mybir.AluOpType.add)
                else:
                    for _ in range(exp_reps):
                        nc.scalar.activation(out=ph, in_=sts[s], func=Exp)
                phs.append(ph)
            return phs

        def emit_st4(qk, g, jb):
            """Four row-tiled score matmuls (4 banks) + one 2048-wide exp."""
            st4 = stpool.tile([128, 4, 512], F32, tag="st", name="st")
            for _ in range(sc_reps):
                for hp in range(4):
                    nc.tensor.matmul(
                        st4[:, hp, :],
                        qk[2 + g][32 * hp:32 * hp + 32, 128 * jb:128 * jb + 128],
                        qk[g][32 * hp:32 * hp + 32, :], start=True, stop=True,
                        tile_position=(32 * hp, 0))
            ph = ppool.tile([128, 4, 512], BF16, tag="ph", name="ph")
            for _ in range(exp_reps):
                nc.scalar.activation(out=ph, in_=st4, func=Exp)
            return ph

        def emit_avdn(vaug, phs, avdn, g, jb):
            """Col-tiled AV + denominator matmuls for all 4 heads of (g, jb)."""
            av, dn = avdn
            if set4:
                mov = [phs[:, hp, :] for hp in range(4)]
            else:
                mov = [phs[hp // 2][:, hp % 2, :] for hp in range(4)]
            for r in range(av_reps):
                for hp in range(4):
                    nc.tensor.matmul(
                        av[32 * hp:32 * hp + 32, :],
                        vaug[:, jb, 4 * g + hp, :],
                        mov[hp],
                        start=(jb == 0 and r == 0),
                        stop=(jb == 3 and r == av_reps - 1),
                        tile_position=(0, 32 * hp), skip_group_check=True)
            if no_dn and jb != 0:
                return
            for r in range(av_reps):
                for hp in range(4):
                    nc.tensor.matmul(
                        dn[32 * hp:32 * hp + 32, :], ones,
                        mov[hp],
                        start=(jb == 0 and r == 0),
                        stop=((jb == 3 or no_dn) and r == av_reps - 1),
                        tile_position=(0, 32 * hp), skip_group_check=True)

        def emit_group_tail(at, avdn, g):
            av, dn = avdn
            rc = recpool.tile([128, 512], F32, tag="rc", name="rc")
            nc.vector.reciprocal(rc, dn)
            nc.vector.tensor_mul(at[:, g, :], av, rc)

        splitproj = os.environ.get("KERNEL_SPLITPROJ", "1") == "1"

        def emit_proj_stage(iv, at, fin, mb):
            ps = mmout.tile([128, 512], F32, tag="mm", name="pps")
            for cb in range(2):
                nc.tensor.matmul(
                    ps, wp_sb[:, cb, 128 * mb:128 * mb + 128],
                    at[:, cb, :], start=(cb == 0), stop=(cb == 1))
            if evact:
                nc.scalar.activation(out=fin[:, mb, :], in_=ps, func=Ident,
                                     bias=bp_sb[:, mb:mb + 1], scale=1.0)
            else:
                nc.vector.tensor_scalar_add(fin[:, mb, :], ps,
                                            bp_sb[:, mb:mb + 1])
            if mb == 1:
                nc.sync.dma_start(out=ot_d[iv], in_=fin)

        def emit_proj(iv, at):
            fin = finpool.tile([128, 2, 512], BF16, tag="fin", name="fin")
            for mb in range(2):
                ps = mmout.tile([128, 512], F32, tag="mm", name="pps")
                for cb in range(2):
                    nc.tensor.matmul(
                        ps, wp_sb[:, cb, 128 * mb:128 * mb + 128],
                        at[:, cb, :], start=(cb == 0), stop=(cb == 1))
                if evact:
                    nc.scalar.activation(out=fin[:, mb, :], in_=ps,
                                         func=Ident, bias=bp_sb[:, mb:mb + 1],
                                         scale=1.0)
                else:
                    nc.vector.tensor_scalar_add(fin[:, mb, :], ps,
                                                bp_sb[:, mb:mb + 1])
            nc.sync.dma_start(out=ot_d[iv], in_=fin)

        defer = os.environ.get("KERNEL_DEFER", "1") == "1"

        pend_depth = int(os.environ.get("KERNEL_PEND", "1"))
        proju = int(os.environ.get("KERNEL_PROJU", "1"))
        tailpri = os.environ.get("KERNEL_TAILPRI", "0") == "1"

        def full_pass():
            chunks, state = make_qkv(0)
            for c in chunks:
                c()
            pend = []   # (ph_entry, vaug, at, avdn_key, g, jb)
            avdns = {}
            proj_q = []

            def pop_pend():
                ph, pvaug, pat, key, pg, pjb = pend.pop(0)
                if pjb == 0:
                    av = avpool.tile([128, 512], F32, tag="av", name="av")
                    dn = avpool.tile([128, 512], F32, tag="dn", name="dn")
                    avdns[key] = (av, dn)
                emit_avdn(pvaug, ph, avdns[key], pg, pjb)
                if pjb == 3:
                    if tailpri:
                        with tc.high_priority():
                            emit_group_tail(pat, avdns.pop(key), pg)
                    else:
                        emit_group_tail(pat, avdns.pop(key), pg)

            for w in range(n_windows):
                qk, vaug = state["qk"], state["vaug"]
                at = atpool.tile([128, 2, 512], BF16, tag="at", name="at")
                if w + 1 < n_windows:
                    next_chunks, state = make_qkv(w + 1)
                else:
                    next_chunks = []
                ci = 0
                proj2 = None
                for u in range(8):
                    g, jb = u // 4, u % 4
                    if set4:
                        ph = emit_st4(qk, g, jb)
                    elif os.environ.get("KERNEL_STPAIR", "1") == "1":
                        ph = emit_st_pair(qk, g, jb)
                    else:
                        ph = [emit_st(qk, g, jb, 0), emit_st(qk, g, jb, 1)]
                    pend.append((ph, vaug, at, (w, g), g, jb))
                    while len(pend) > pend_depth:
                        pop_pend()
                    if u == proju and proj_q:
                        if splitproj:
                            piv, pat = proj_q.pop(0)
                            pfin = finpool.tile([128, 2, 512], BF16,
                                                tag="fin", name="fin")
                            emit_proj_stage(piv, pat, pfin, 0)
                            proj2 = (piv, pat, pfin)
                        else:
                            emit_proj(*proj_q.pop(0))
                    if (u == (proju + 2 if os.environ.get("KERNEL_PROJ3", "0") == "1"
                              else proju + 1) and proj2 is not None):
                        emit_proj_stage(*proj2, 1)
                        proj2 = None
                    if (ci < len(next_chunks)
                            and not (splitproj and u == proju)
                            and not (schmm and jb in schraud_jbs)):
                        next_chunks[ci]()
                        ci += 1
                while ci < len(next_chunks):
                    next_chunks[ci]()
                    ci += 1
                if defer and w + 1 < n_windows:
                    proj_q.append((w, at))
                else:
                    while pend:
                        pop_pend()
                    while proj_q:
                        emit_proj(*proj_q.pop(0))
                    emit_proj(w, at)

        body_passes = int(os.environ.get("KERNEL_BODY_PASSES", "1"))
        if repeat > 1:
            def rep_body(r):
                for _ in range(body_passes):
                    full_pass()
            tc.For_i_unrolled(0, repeat, 1, rep_body, max_unroll=1)
        else:
            full_pass()

    nc.compile()
    return nc


def prep_inputs(x, qkv_w, qkv_b, proj_w, proj_b, n_windows_per_core=W,
                n_cores=N_CORES):
    """Shard + lay out inputs for the per-core DRAM parameters."""
    x = np.asarray(x, dtype=np.float32)
    qkv_w = np.asarray(qkv_w, dtype=np.float32)
    qkv_b = np.asarray(qkv_b, dtype=np.float32)
    proj_w = np.asarray(proj_w, dtype=np.float32)
    proj_b = np.asarray(proj_b, dtype=np.float32)

    sc = HD ** -0.5
    qkv_w_s = qkv_w.copy()
    qkv_w_s[:C] *= sc
    qkv_b_s = qkv_b.copy()
    qkv_b_s[:C] *= sc

    # [768, 256] rows: q feats (head-major), k feats, v feats
    wq = np.ascontiguousarray(
        qkv_w_s.reshape(768, 2, 128).transpose(2, 1, 0)).astype(
            ml_dtypes.bfloat16)
    bq = np.ascontiguousarray(qkv_b_s[:512].reshape(4, 128).T)
    bv = np.ascontiguousarray(np.broadcast_to(qkv_b[2 * C:], (128, C)))
    wp = np.ascontiguousarray(
        proj_w.reshape(C, 2, 128).transpose(2, 1, 0)).astype(ml_dtypes.bfloat16)
    proj_b_eff = proj_b
    if os.environ.get("KERNEL_VFOLD", "0") == "1":
        proj_b_eff = proj_b + proj_w @ qkv_b[2 * C:]
    bp = np.ascontiguousarray(proj_b_eff.reshape(2, 128).T)

    in_maps = []
    for c in range(n_cores):
        xs = x[c * n_windows_per_core:(c + 1) * n_windows_per_core]
        xt = np.ascontiguousarray(
            xs.reshape(n_windows_per_core, N, 2, 128).transpose(0, 3, 2, 1)
        ).astype(ml_dtypes.bfloat16)
        in_maps.append(
            {"xt": xt, "wq": wq, "bq": bq, "bv": bv, "wp": wp, "bp": bp})
    return in_maps


def assemble_output(results, n_windows_per_core=W, n_cores=N_CORES):
    outs = []
    for c in range(n_cores):
        ot = results[c]["ot"]  # [W, 128, 2, 512] bf16
        y = np.asarray(ot, dtype=np.float32).transpose(0, 3, 2, 1).reshape(
            n_windows_per_core, N, C)
        outs.append(y)
    return np.ascontiguousarray(np.concatenate(outs, axis=0), dtype=np.float32)


_NC_CACHE = {}
LAST_EXEC_TIME_NS = None


def kernel(x, qkv_w, qkv_b, proj_w, proj_b):
    global LAST_EXEC_TIME_NS
    from concourse.bass_utils import run_bass_kernel_spmd

    if "nc" not in _NC_CACHE:
        _NC_CACHE["nc"] = build_nc(W, repeat=1)
    nc = _NC_CACHE["nc"]

    in_maps = prep_inputs(x, qkv_w, qkv_b, proj_w, proj_b)
    res = run_bass_kernel_spmd(nc, in_maps, core_ids=list(range(N_CORES)))
    LAST_EXEC_TIME_NS = res.exec_time_ns
    return assemble_output(res.results)



# revision 17
# speedup vs baseline: 1.3448x; 1.3448x over previous
"""Windowed multi-head attention (B=128 windows, N=512, C=256, H=8) on 8 TRN2 NeuronCores.

Strategy: data-parallel over windows (16 per core). The softmax exp on the
ScalarE (ACT) engine is the hard bottleneck (8 heads x 512x512 exps per
window = 16384 ACT lane-cycles @1.2GHz ~ 13.7us/window), so the kernel is
built so ACT does *only* exp and is never starved:

  - q/k packed 4 heads per 128-partition block; scores use 4x row-tiled
    matmuls (tile_position=(32h',0), contraction=head_dim=32) so 2-4 score
    matmuls run concurrently in the PE array.
  - exp consumes 2-bank PSUM score sets [128, 2x512] in single ACT calls.
  - AV uses 4x column tiling (stationary = V_h [j,32], moving = P_h) so the
    attention output lands *pre-transposed* [d, q] in one PSUM bank -- no
    PE transposes. A second col-tiled pass with an all-ones [128,32]
    stationary produces the softmax denominator replicated across each
    32-partition group, so normalization is a full-width DVE recip+mul.
  - all PSUM evacuations (qkv bias-add, v bias-add, normalize, proj bias)
    run on VectorE; outputs are written bf16 to halve the store DMA.

Per window, 8 software-pipeline units (g group of 4 heads, jb key block):
unit = 4 row-tiled score matmuls (two 2-bank PSUM sets) + 2 exp calls;
the previous unit's 8 AV/denominator matmuls trail one unit behind
(batched to halve PE mode-switch drains); the next window's qkv matmuls
are interleaved one chunk per unit; the last unit's AV work and the
projection trail into the NEXT window (KERNEL_DEFER) so ACT never idles
at window boundaries.  KERNEL_SCHRAUD=n offloads n units' exp to the
VectorE as a one-instruction Schraudolph bf16 exp (tensor_scalar
round(s*128/ln2 + 16256) -> int16, bitcast bf16), splitting the exp
dependency ladder across two engines at a small accuracy cost.
KERNEL_STPAIR (default on) emits all 4 score matmuls of a unit before
its two exp calls so the second exp reaches the ACT queue head with its
semaphore wait already satisfied.

Measured HW facts (8-core repeat-loop bench, ~±15us/pass run noise):
  - baseline ~392-412us/pass (16 windows/core); ACT exp work is ~1:1 on
    the critical path (EXP_REPS=2 adds 94% of the added ACT time); DVE
    ops cost ~1.8x nominal on the critical path (post-op DRAIN).
  - removing 24/32 denominator matmuls (NO_DN=1) does NOT speed it up:
    the PE AV/dn work trails in slack; only the score->exp chain and
    the two softmax engines are critical.
  - every engine-reassignment tried is worse: SCHRAUD=0 +80us,
    SCHRAUD=8 +214us, EVACT=1 +34us, SET4=1 (single-buffered st) +60us,
    PEND=2 +40us.  SCHRAUD=2 with evacs on DVE is the balance point.
  - PSUM is exactly full (st 2x2 + av/dn 2 + mmout 2 = 8 banks); a 3rd
    st set (which would let ACT and DVE exp overlap across units) does
    not fit, which caps further gains from engine overlap.
"""
import os
import sys

sys.path.insert(0, "/opt/trn_rl_repo")

import numpy as np
import ml_dtypes
from contextlib import ExitStack

N_CORES = 8
B, N, C = 128, 512, 256
H, HD = 8, 32
W = B // N_CORES  # windows per core


def build_nc(n_windows=W, repeat=None):
    import concourse.bass as bass
    import concourse.tile as tile
    from concourse import bacc, mybir

    if repeat is None:
        repeat = int(os.environ.get("KERNEL_REPEAT", "1"))

    F32 = mybir.dt.float32
    BF16 = mybir.dt.bfloat16
    MMDT = BF16
    Exp = mybir.ActivationFunctionType.Exp
    Ident = mybir.ActivationFunctionType.Identity
    # route qkv/proj bias-add evacuations to ACT (Identity shares the exp
    # table set) to free DVE for more Schraudolph exp calls
    evact = os.environ.get("KERNEL_EVACT", "0") == "1"
    # fold the V bias into the proj bias host-side (softmax rows sum to 1,
    # so V-bias contributes exactly proj_w @ v_bias to the output); the V
    # evacuation becomes a pure copy, optionally on ACT
    vfold = os.environ.get("KERNEL_VFOLD", "0") == "1"
    vevac_act = os.environ.get("KERNEL_VEVAC", "dve") == "act"

    nc = bacc.Bacc("TRN2", target_bir_lowering=False, debug=False,
                   num_devices=N_CORES)
    xt_d = nc.dram_tensor("xt", [n_windows, 128, 2, 512], MMDT,
                          kind="ExternalInput").ap()
    wq_d = nc.dram_tensor("wq", [128, 2, 768], MMDT, kind="ExternalInput").ap()
    bq_d = nc.dram_tensor("bq", [128, 4], F32, kind="ExternalInput").ap()
    bv_d = nc.dram_tensor("bv", [128, 256], F32, kind="ExternalInput").ap()
    wp_d = nc.dram_tensor("wp", [128, 2, 256], BF16, kind="ExternalInput").ap()
    bp_d = nc.dram_tensor("bp", [128, 2], F32, kind="ExternalInput").ap()
    ot_d = nc.dram_tensor("ot", [n_windows, 128, 2, 512], BF16,
                          kind="ExternalOutput").ap()

    with tile.TileContext(nc) as tc, ExitStack() as ctx:
        big = os.environ.get("KERNEL_POOLBIG", "0") == "1"
        persist = ctx.enter_context(tc.tile_pool(name="persist", bufs=1))
        xpool = ctx.enter_context(tc.tile_pool(name="xpool", bufs=4 if big else 3))
        qkpool = ctx.enter_context(tc.tile_pool(name="qkpool", bufs=2))
        vpool = ctx.enter_context(tc.tile_pool(name="vpool", bufs=3 if big else 2))
        _pend_bufs = 2 * (int(os.environ.get("KERNEL_PEND", "1")) - 1)
        ppool = ctx.enter_context(tc.tile_pool(
            name="ppool", bufs=(10 if big else 6) + _pend_bufs))
        recpool = ctx.enter_context(tc.tile_pool(name="recpool", bufs=3 if big else 2))
        atpool = ctx.enter_context(tc.tile_pool(name="atpool", bufs=2))
        finpool = ctx.enter_context(tc.tile_pool(name="finpool", bufs=4 if big else 3))
        stpool = ctx.enter_context(tc.tile_pool(
            name="stpool",
            bufs=1 if os.environ.get("KERNEL_SET4", "0") == "1" else 2,
            space="PSUM"))
        avpool = ctx.enter_context(tc.tile_pool(name="avpool", bufs=1, space="PSUM"))
        mmout = ctx.enter_context(tc.tile_pool(name="mmout", bufs=2, space="PSUM"))

        wq_sb = persist.tile([128, 2, 768], MMDT, tag="wq")
        nc.sync.dma_start(out=wq_sb, in_=wq_d)
        bq_sb = persist.tile([128, 4], F32, tag="bq")
        nc.sync.dma_start(out=bq_sb, in_=bq_d)
        bv_sb = persist.tile([128, 256], F32, tag="bv")
        nc.sync.dma_start(out=bv_sb, in_=bv_d)
        wp_sb = persist.tile([128, 2, 256], BF16, tag="wp")
        nc.sync.dma_start(out=wp_sb, in_=wp_d)
        bp_sb = persist.tile([128, 2], F32, tag="bp")
        nc.sync.dma_start(out=bp_sb, in_=bp_d)
        ones = persist.tile([128, 32], BF16, tag="ones")
        nc.gpsimd.memset(ones, 1.0)

        def make_qkv(iv):
            """Emit qkv for window iv as a list of chunk closures."""
            state = {"qk": []}

            def c0():
                xw = xpool.tile([128, 2, 512], MMDT, tag="xw", name="xw")
                nc.sync.dma_start(out=xw, in_=xt_d[iv])
                state["xw"] = xw
                state["vaug"] = vpool.tile([128, 4, 8, 32], BF16, tag="vaug",
                                           name="vaug")

            def cmb(mb):
                def f():
                    ps = mmout.tile([128, 512], F32, tag="mm", name="qkps")
                    xw = state["xw"]
                    for cb in range(2):
                        nc.tensor.matmul(
                            ps, wq_sb[:, cb, 128 * mb:128 * mb + 128],
                            xw[:, cb, :], start=(cb == 0), stop=(cb == 1))
                    qkt = qkpool.tile([128, 512], BF16, tag=f"qk{mb}",
                                      name=f"qk{mb}")
                    if evact:
                        nc.scalar.activation(out=qkt, in_=ps, func=Ident,
                                             bias=bq_sb[:, mb:mb + 1],
                                             scale=1.0)
                    else:
                        nc.vector.tensor_scalar_add(qkt, ps,
                                                    bq_sb[:, mb:mb + 1])
                    state["qk"].append(qkt)
                return f

            def cv(tp):
                def f():
                    ps = mmout.tile([128, 512], F32, tag="mm", name="vps")
                    xw, vaug = state["xw"], state["vaug"]
                    for half in range(2):
                        for cb in range(2):
                            nc.tensor.matmul(
                                ps[:, 256 * half:256 * half + 256],
                                xw[:, cb, 128 * (2 * tp + half):
                                   128 * (2 * tp + half) + 128],
                                wq_sb[:, cb, 512:768],
                                start=(cb == 0), stop=(cb == 1))
                    dst = vaug[:, 2 * tp:2 * tp + 2, :, :]
                    src = ps.rearrange("p (t h d) -> p t h d", t=2, h=8)
                    if vfold:
                        if vevac_act:
                            nc.scalar.copy(dst, src)
                        else:
                            nc.vector.tensor_copy(dst, src)
                    else:
                        bvb = bass.AP(tensor=bv_sb.tensor, offset=bv_sb.offset,
                                      ap=[[bv_sb.ap[0][0], 128], [0, 2],
                                          [32, 8], [1, 32]])
                        nc.vector.tensor_add(dst, src, bvb)
                return f

            return [c0, cmb(0), cmb(1), cmb(2), cmb(3), cv(0), cv(1)], state

        sc_reps = int(os.environ.get("KERNEL_SC_REPS", "1"))
        av_reps = int(os.environ.get("KERNEL_AV_REPS", "1"))
        exp_reps = int(os.environ.get("KERNEL_EXP_REPS", "1"))
        no_dn = os.environ.get("KERNEL_NO_DN", "0") == "1"
        set4 = os.environ.get("KERNEL_SET4", "0") == "1"
        # units per window whose exp runs on DVE (Schraudolph bf16) instead
        # of ACT: splits the exp dependency ladder across two engines
        schraud = int(os.environ.get("KERNEL_SCHRAUD", "2"))
        schraud_jbs = [[], [2], [1, 3], [1, 2, 3], [0, 1, 2, 3]][schraud // 2]
        SCH_A = float(128.0 / np.log(2.0))
        SCH_B = 16256.0
        I16 = mybir.dt.int16

        def emit_st(qk, g, jb, s):
            """Two row-tiled score matmuls + one exp call for head pair."""
            st2 = stpool.tile([128, 2, 512], F32, tag="st", name="st")
            for _ in range(sc_reps):
                for i in range(2):
                    hp = 2 * s + i
                    nc.tensor.matmul(
                        st2[:, i, :],
                        qk[2 + g][32 * hp:32 * hp + 32, 128 * jb:128 * jb + 128],
                        qk[g][32 * hp:32 * hp + 32, :], start=True, stop=True,
                        tile_position=(32 * hp, 0))
            ph = ppool.tile([128, 2, 512], BF16, tag="ph", name="ph")
            if jb in schraud_jbs:
                nc.vector.tensor_scalar(
                    out=ph.bitcast(I16), in0=st2, scalar1=SCH_A, scalar2=SCH_B,
                    op0=mybir.AluOpType.mult, op1=mybir.AluOpType.add)
            else:
                for _ in range(exp_reps):
                    nc.scalar.activation(out=ph, in_=st2, func=Exp)
            return ph

        schmm = os.environ.get("KERNEL_SCHMM", "0") == "1"

        def emit_st_pair(qk, g, jb):
            """All 4 score matmuls first, then both exp calls — halves the
            number of dependent sem hops on the exp stream (the second exp's
            wait is already satisfied when it reaches the queue head).

            KERNEL_SCHMM=1: Schraudolph (DVE) units take their score tiles
            from the mmout pool (1-bank granularity) instead of stpool, so
            the ACT units' st-set ping-pong never passes through a DVE unit
            and the ACT exp stream can run ahead across it."""
            if schmm and jb in schraud_jbs:
                sts = []
                for hp in range(4):
                    st1 = mmout.tile([128, 512], F32, tag="mm", name="sst")
                    for _ in range(sc_reps):
                        nc.tensor.matmul(
                            st1,
                            qk[2 + g][32 * hp:32 * hp + 32,
                                      128 * jb:128 * jb + 128],
                            qk[g][32 * hp:32 * hp + 32, :], start=True,
                            stop=True, tile_position=(32 * hp, 0))
                    sts.append(st1)
                phs = []
                for s in range(2):
                    ph = ppool.tile([128, 2, 512], BF16, tag="ph", name="ph")
                    for i in range(2):
                        nc.vector.tensor_scalar(
                            out=ph[:, i, :].bitcast(I16), in0=sts[2 * s + i],
                            scalar1=SCH_A, scalar2=SCH_B,
                            op0=mybir.AluOpType.mult,
                            op1=mybir.AluOpType.add)
                    phs.append(ph)
                return phs
            sts = []
            for s in range(2):
                st2 = stpool.tile([128, 2, 512], F32, tag="st", name="st")
                for _ in range(sc_reps):
                    for i in range(2):
                        hp = 2 * s + i
                        nc.tensor.matmul(
                            st2[:, i, :],
                            qk[2 + g][32 * hp:32 * hp + 32,
                                      128 * jb:128 * jb + 128],
                            qk[g][32 * hp:32 * hp + 32, :], start=True,
                            stop=True, tile_position=(32 * hp, 0))
                sts.append(st2)
            phs = []
            for s in range(2):
                ph = ppool.tile([128, 2, 512], BF16, tag="ph", name="ph")
                if jb in schraud_jbs:
                    nc.vector.tensor_scalar(
                        out=ph.bitcast(I16), in0=sts[s], scalar1=SCH_A,
                        scalar2=SCH_B, op0=mybir.AluOpType.mult,
                        op1=mybir.AluOpType.add)
                else:
                    for _ in range(exp_reps):
                        nc.scalar.activation(out=ph, in_=sts[s], func=Exp)
                phs.append(ph)
            return phs

        def emit_st4(qk, g, jb):
            """Four row-tiled score matmuls (4 banks) + one 2048-wide exp."""
            st4 = stpool.tile([128, 4, 512], F32, tag="st", name="st")
            for _ in range(sc_reps):
                for hp in range(4):
                    nc.tensor.matmul(
                        st4[:, hp, :],
                        qk[2 + g][32 * hp:32 * hp + 32, 128 * jb:128 * jb + 128],
                        qk[g][32 * hp:32 * hp + 32, :], start=True, stop=True,
                        tile_position=(32 * hp, 0))
            ph = ppool.tile([128, 4, 512], BF16, tag="ph", name="ph")
            for _ in range(exp_reps):
                nc.scalar.activation(out=ph, in_=st4, func=Exp)
            return ph

        def emit_avdn(vaug, phs, avdn, g, jb):
            """Col-tiled AV + denominator matmuls for all 4 heads of (g, jb)."""
            av, dn = avdn
            if set4:
                mov = [phs[:, hp, :] for hp in range(4)]
            else:
                mov = [phs[hp // 2][:, hp % 2, :] for hp in range(4)]
            for r in range(av_reps):
                for hp in range(4):
                    nc.tensor.matmul(
                        av[32 * hp:32 * hp + 32, :],
                        vaug[:, jb, 4 * g + hp, :],
                        mov[hp],
                        start=(jb == 0 and r == 0),
                        stop=(jb == 3 and r == av_reps - 1),
                        tile_position=(0, 32 * hp), skip_group_check=True)
            if no_dn and jb != 0:
                return
            for r in range(av_reps):
                for hp in range(4):
                    nc.tensor.matmul(
                        dn[32 * hp:32 * hp + 32, :], ones,
                        mov[hp],
                        start=(jb == 0 and r == 0),
                        stop=((jb == 3 or no_dn) and r == av_reps - 1),
                        tile_position=(0, 32 * hp), skip_group_check=True)

        def emit_group_tail(at, avdn, g):
            av, dn = avdn
            rc = recpool.tile([128, 512], F32, tag="rc", name="rc")
            if os.environ.get("KERNEL_RECFAST", "1") == "1":
                # single-instruction approximate reciprocal (~51 ULP, ~5x
                # faster than the bit-exact ~6 cpe iterative divide); the
                # denominator error (~3e-6 rel) is negligible vs the bf16
                # score/exp path
                nc.vector.reciprocal_approx_fast(rc, dn)
            else:
                nc.vector.reciprocal(rc, dn)
            nc.vector.tensor_mul(at[:, g, :], av, rc)

        splitproj = os.environ.get("KERNEL_SPLITPROJ", "1") == "1"

        def emit_proj_stage(iv, at, fin, mb):
            ps = mmout.tile([128, 512], F32, tag="mm", name="pps")
            for cb in range(2):
                nc.tensor.matmul(
                    ps, wp_sb[:, cb, 128 * mb:128 * mb + 128],
                    at[:, cb, :], start=(cb == 0), stop=(cb == 1))
            if evact:
                nc.scalar.activation(out=fin[:, mb, :], in_=ps, func=Ident,
                                     bias=bp_sb[:, mb:mb + 1], scale=1.0)
            else:
                nc.vector.tensor_scalar_add(fin[:, mb, :], ps,
                                            bp_sb[:, mb:mb + 1])
            if mb == 1:
                nc.sync.dma_start(out=ot_d[iv], in_=fin)

        def emit_proj(iv, at):
            fin = finpool.tile([128, 2, 512], BF16, tag="fin", name="fin")
            for mb in range(2):
                ps = mmout.tile([128, 512], F32, tag="mm", name="pps")
                for cb in range(2):
                    nc.tensor.matmul(
                        ps, wp_sb[:, cb, 128 * mb:128 * mb + 128],
                        at[:, cb, :], start=(cb == 0), stop=(cb == 1))
                if evact:
                    nc.scalar.activation(out=fin[:, mb, :], in_=ps,
                                         func=Ident, bias=bp_sb[:, mb:mb + 1],
                                         scale=1.0)
                else:
                    nc.vector.tensor_scalar_add(fin[:, mb, :], ps,
                                                bp_sb[:, mb:mb + 1])
            nc.sync.dma_start(out=ot_d[iv], in_=fin)

        defer = os.environ.get("KERNEL_DEFER", "1") == "1"

        pend_depth = int(os.environ.get("KERNEL_PEND", "1"))
        proju = int(os.environ.get("KERNEL_PROJU", "1"))
        tailpri = os.environ.get("KERNEL_TAILPRI", "0") == "1"

        def full_pass():
            chunks, state = make_qkv(0)
            for c in chunks:
                c()
            pend = []   # (ph_entry, vaug, at, avdn_key, g, jb)
            avdns = {}
            proj_q = []

            def pop_pend():
                ph, pvaug, pat, key, pg, pjb = pend.pop(0)
                if pjb == 0:
                    av = avpool.tile([128, 512], F32, tag="av", name="av")
                    dn = avpool.tile([128, 512], F32, tag="dn", name="dn")
                    avdns[key] = (av, dn)
                emit_avdn(pvaug, ph, avdns[key], pg, pjb)
                if pjb == 3:
                    if tailpri:
                        with tc.high_priority():
                            emit_group_tail(pat, avdns.pop(key), pg)
                    else:
                        emit_group_tail(pat, avdns.pop(key), pg)

            for w in range(n_windows):
                qk, vaug = state["qk"], state["vaug"]
                at = atpool.tile([128, 2, 512], BF16, tag="at", name="at")
                if w + 1 < n_windows:
                    next_chunks, state = make_qkv(w + 1)
                else:
                    next_chunks = []
                ci = 0
                proj2 = None
                for u in range(8):
                    g, jb = u // 4, u % 4
                    if set4:
                        ph = emit_st4(qk, g, jb)
                    elif os.environ.get("KERNEL_STPAIR", "1") == "1":
                        ph = emit_st_pair(qk, g, jb)
                    else:
                        ph = [emit_st(qk, g, jb, 0), emit_st(qk, g, jb, 1)]
                    pend.append((ph, vaug, at, (w, g), g, jb))
                    while len(pend) > pend_depth:
                        pop_pend()
                    if u == proju and proj_q:
                        if splitproj:
                            piv, pat = proj_q.pop(0)
                            pfin = finpool.tile([128, 2, 512], BF16,
                                                tag="fin", name="fin")
                            emit_proj_stage(piv, pat, pfin, 0)
                            proj2 = (piv, pat, pfin)
                        else:
                            emit_proj(*proj_q.pop(0))
                    if (u == (proju + 2 if os.environ.get("KERNEL_PROJ3", "0") == "1"
                              else proju + 1) and proj2 is not None):
                        emit_proj_stage(*proj2, 1)
                        proj2 = None
                    if (ci < len(next_chunks)
                            and not (splitproj and u == proju)
                            and not (schmm and jb in schraud_jbs)):
                        next_chunks[ci]()
                        ci += 1
                while ci < len(next_chunks):
                    next_chunks[ci]()
                    ci += 1
                if defer and w + 1 < n_windows:
                    proj_q.append((w, at))
                else:
                    while pend:
                        pop_pend()
                    while proj_q:
                        emit_proj(*proj_q.pop(0))
                    emit_proj(w, at)

        body_passes = int(os.environ.get("KERNEL_BODY_PASSES", "1"))
        if repeat > 1:
            def rep_body(r):
                for _ in range(body_passes):
                    full_pass()
            tc.For_i_unrolled(0, repeat, 1, rep_body, max_unroll=1)
        else:
            full_pass()

    nc.compile()
    return nc


def prep_inputs(x, qkv_w, qkv_b, proj_w, proj_b, n_windows_per_core=W,
                n_cores=N_CORES):
    """Shard + lay out inputs for the per-core DRAM parameters."""
    x = np.asarray(x, dtype=np.float32)
    qkv_w = np.asarray(qkv_w, dtype=np.float32)
    qkv_b = np.asarray(qkv_b, dtype=np.float32)
    proj_w = np.asarray(proj_w, dtype=np.float32)
    proj_b = np.asarray(proj_b, dtype=np.float32)

    sc = HD ** -0.5
    qkv_w_s = qkv_w.copy()
    qkv_w_s[:C] *= sc
    qkv_b_s = qkv_b.copy()
    qkv_b_s[:C] *= sc

    # [768, 256] rows: q feats (head-major), k feats, v feats
    wq = np.ascontiguousarray(
        qkv_w_s.reshape(768, 2, 128).transpose(2, 1, 0)).astype(
            ml_dtypes.bfloat16)
    bq = np.ascontiguousarray(qkv_b_s[:512].reshape(4, 128).T)
    bv = np.ascontiguousarray(np.broadcast_to(qkv_b[2 * C:], (128, C)))
    wp = np.ascontiguousarray(
        proj_w.reshape(C, 2, 128).transpose(2, 1, 0)).astype(ml_dtypes.bfloat16)
    proj_b_eff = proj_b
    if os.environ.get("KERNEL_VFOLD", "0") == "1":
        proj_b_eff = proj_b + proj_w @ qkv_b[2 * C:]
    bp = np.ascontiguousarray(proj_b_eff.reshape(2, 128).T)

    in_maps = []
    for c in range(n_cores):
        xs = x[c * n_windows_per_core:(c + 1) * n_windows_per_core]
        xt = np.ascontiguousarray(
            xs.reshape(n_windows_per_core, N, 2, 128).transpose(0, 3, 2, 1)
        ).astype(ml_dtypes.bfloat16)
        in_maps.append(
            {"xt": xt, "wq": wq, "bq": bq, "bv": bv, "wp": wp, "bp": bp})
    return in_maps


def assemble_output(results, n_windows_per_core=W, n_cores=N_CORES):
    outs = []
    for c in range(n_cores):
        ot = results[c]["ot"]  # [W, 128, 2, 512] bf16
        y = np.asarray(ot, dtype=np.float32).transpose(0, 3, 2, 1).reshape(
            n_windows_per_core, N, C)
        outs.append(y)
    return np.ascontiguousarray(np.concatenate(outs, axis=0), dtype=np.float32)


_NC_CACHE = {}
LAST_EXEC_TIME_NS = None


def kernel(x, qkv_w, qkv_b, proj_w, proj_b):
    global LAST_EXEC_TIME_NS
    from concourse.bass_utils import run_bass_kernel_spmd

    if "nc" not in _NC_CACHE:
        _NC_CACHE["nc"] = build_nc(W, repeat=1)
    nc = _NC_CACHE["nc"]

    in_maps = prep_inputs(x, qkv_w, qkv_b, proj_w, proj_b)
    res = run_bass_kernel_spmd(nc, in_maps, core_ids=list(range(N_CORES)))
    LAST_EXEC_TIME_NS = res.exec_time_ns
    return assemble_output(res.results)

